# revision 14
# baseline (speedup 1.0000x reference)
"""Trainium2 Bass kernel for blocked (compressed) multi-head attention.

Problem (hardcoded shapes):
    src [4, 4096, 1024] f32, H = 8 heads, dk = 128, local attention in
    blocks of 64 tokens, projections Wq/Wk/Wv/Wo [1024,1024] + biases.

Strategy:
    - 8-way data parallel over the 16384 tokens (2048 tokens/core; block and
      batch boundaries align, so cores are fully independent).
    - Host pre-transposes src to [D, T] and casts weights/src to bf16.
    - Per core, tokens are processed in chunks of 512; all matmuls in bf16
      with fp32 PSUM accumulation:
        qT/kT (d-major) and v (token-major) projections;
        per 128-token block pair and group of 4 heads: one rank-2 "mask"
        matmul writes -30/scale into cross-block score entries, then 4
        scores matmuls accumulate q.k; one Exp on ACT; softmax denominator
        via one DVE reduction; normalization as one broadcasted DVE multiply;
        probs transposed per head on the PE; PV matmul with token-major v
        stationary gives attn^T (d-major); per-pair output projection
        out = attn^T.T @ Wo + bo in token-major, DMA'd out as fp32.
"""

import numpy as np
import ml_dtypes
from contextlib import ExitStack

import sys
import types

# Defensive: bass_utils imports antenv.axon_hooks when BASS_TRACE is set in
# the environment; provide a no-op hook module if the package is absent.
try:
    import antenv.axon_hooks  # noqa: F401
except ImportError:
    _anthooks = types.ModuleType("antenv.axon_hooks")
    _anthooks.get_axon_ntff_profile_hook = lambda: None
    _anthooks.set_axon_ntff_profile_hook = lambda h: None
    _antenv = sys.modules.setdefault("antenv", types.ModuleType("antenv"))
    _antenv.axon_hooks = _anthooks
    sys.modules.setdefault("antenv.axon_hooks", _anthooks)

import concourse.bass as bass
import concourse.tile as tile
from concourse import bacc, mybir
from concourse.bass_utils import run_bass_kernel_spmd

N_CORES = 8
B, S, D = 4, 4096, 1024
H, DK, BLOCK = 8, 128, 64
T_TOTAL = B * S
T_CORE = T_TOTAL // N_CORES   # 2048
NJ = D // 128                 # 8 column/row tiles of the weights
SCALE = 1.0 / float(np.sqrt(DK))
MASK_RAW = -30.0 * float(np.sqrt(DK))   # becomes -30 after activation scale

BF16 = mybir.dt.bfloat16
F32 = mybir.dt.float32
Exp = mybir.ActivationFunctionType.Exp
Copy = mybir.ActivationFunctionType.Copy
Mult = mybir.AluOpType.mult
USE_DMA_TR = False


def build_nc(t_core: int = T_CORE, chunk: int = 512) -> bacc.Bacc:
    assert t_core % chunk == 0 and chunk % 128 == 0
    nchunk = t_core // chunk
    pairs = chunk // 128            # 128-token block pairs per chunk
    nhalf = D // 512                # 512-wide output column groups

    nc = bacc.Bacc("TRN2", target_bir_lowering=False, debug=False,
                   num_devices=N_CORES)

    srct = nc.dram_tensor("srct", [D, t_core], BF16, kind="ExternalInput").ap()
    wq = nc.dram_tensor("wq", [D, D], BF16, kind="ExternalInput").ap()
    wk = nc.dram_tensor("wk", [D, D], BF16, kind="ExternalInput").ap()
    wv = nc.dram_tensor("wv", [D, D], BF16, kind="ExternalInput").ap()
    wo = nc.dram_tensor("wo", [D, D], BF16, kind="ExternalInput").ap()
    bqt = nc.dram_tensor("bqt", [128, NJ], F32, kind="ExternalInput").ap()
    bkt = nc.dram_tensor("bkt", [128, NJ], F32, kind="ExternalInput").ap()
    bo = nc.dram_tensor("bo", [D], F32, kind="ExternalInput").ap()
    maskl = nc.dram_tensor("maskl", [2, 128], BF16, kind="ExternalInput").ap()
    maskr = nc.dram_tensor("maskr", [2, 512], BF16, kind="ExternalInput").ap()
    ident = nc.dram_tensor("ident", [128, 128], BF16, kind="ExternalInput").ap()
    out = nc.dram_tensor("out", [t_core, D], F32, kind="ExternalOutput").ap()

    srct_r = srct.rearrange("(j p) t -> p j t", p=128)
    w_r = {"wq": wq.rearrange("(j p) n -> p j n", p=128),
           "wk": wk.rearrange("(j p) n -> p j n", p=128),
           "wv": wv.rearrange("(j p) n -> p j n", p=128),
           "wo": wo.rearrange("(j p) n -> p j n", p=128)}

    with tile.TileContext(nc) as tc, ExitStack() as ctx:
        const = ctx.enter_context(tc.tile_pool(name="const", bufs=1))
        srcp = ctx.enter_context(tc.tile_pool(name="srcp", bufs=2))
        qkp = ctx.enter_context(tc.tile_pool(name="qkp", bufs=2))
        vp = ctx.enter_context(tc.tile_pool(name="vp", bufs=2))
        attnp = ctx.enter_context(tc.tile_pool(name="attnp", bufs=2))
        smp = ctx.enter_context(tc.tile_pool(name="smp", bufs=8))
        outp = ctx.enter_context(tc.tile_pool(name="outp", bufs=4))
        abufs = 3 if USE_DMA_TR else 2
        ps_proj = ctx.enter_context(tc.tile_pool(name="ps_proj", bufs=2, space="PSUM"))
        ps_sc = ctx.enter_context(tc.tile_pool(name="ps_sc", bufs=abufs, space="PSUM"))
        ps_ao = ctx.enter_context(tc.tile_pool(name="ps_ao", bufs=abufs, space="PSUM"))
        if not USE_DMA_TR:
            ps_tr = ctx.enter_context(tc.tile_pool(name="ps_tr", bufs=2, space="PSUM"))

        def load_w(name):
            tiles = []
            for i in range(NJ):
                t = const.tile([128, D], BF16, tag=f"{name}{i}")
                nc.sync.dma_start(out=t, in_=w_r[name][:, i, :])
                tiles.append(t)
            return tiles

        def load_src_chunk(ci):
            tiles = []
            c0 = ci * chunk
            for i in range(NJ):
                t = srcp.tile([128, chunk], BF16, tag=f"s{i}")
                nc.sync.dma_start(out=t, in_=srct_r[:, i, c0:c0 + chunk])
                tiles.append(t)
            return tiles

        # ---- constants; ordered so the first chunk's work can start early --
        wq_sb = load_w("wq")
        s_next = load_src_chunk(0)
        maskl_sb = const.tile([2, 128], BF16, tag="maskl")
        nc.sync.dma_start(out=maskl_sb, in_=maskl)
        maskr_sb = const.tile([2, 512], BF16, tag="maskr")
        nc.sync.dma_start(out=maskr_sb, in_=maskr)
        ident_sb = const.tile([128, 128], BF16, tag="ident")
        nc.sync.dma_start(out=ident_sb, in_=ident)
        bqt_sb = const.tile([128, NJ], F32, tag="bqt")
        nc.sync.dma_start(out=bqt_sb, in_=bqt)
        bkt_sb = const.tile([128, NJ], F32, tag="bkt")
        nc.sync.dma_start(out=bkt_sb, in_=bkt)
        wk_sb = load_w("wk")
        wv_sb = load_w("wv")
        bo_sb = const.tile([128, D], F32, tag="bo")
        nc.sync.dma_start(out=bo_sb,
                          in_=bass.AP(tensor=bo.tensor, offset=bo.offset,
                                      ap=[[0, 128], [1, D]]))
        wo_sb = load_w("wo")

        for ci in range(nchunk):
            c0 = ci * chunk
            s_sb = s_next

            # ---- q/k projections (d-major) ----
            qt_sb, kt_sb = [], []
            for w_t, bt_sb, dst, nm in ((wq_sb, bqt_sb, qt_sb, "qt"),
                                        (wk_sb, bkt_sb, kt_sb, "kt")):
                for j in range(NJ):
                    acc = ps_proj.tile([128, chunk], F32, tag="acc")
                    for i in range(NJ):
                        nc.tensor.matmul(acc, w_t[i][:, j * 128:(j + 1) * 128],
                                         s_sb[i],
                                         start=(i == 0), stop=(i == NJ - 1))
                    d = qkp.tile([128, chunk], BF16, tag=f"{nm}{j}")
                    nc.vector.tensor_scalar_add(d, acc, bt_sb[:, j:j + 1])
                    dst.append(d)

            # ---- v projection (token-major) ----
            v_sb = []
            for t in range(pairs):
                vt = vp.tile([128, D], BF16, tag=f"v{t}")
                for n in range(nhalf):
                    acc = ps_proj.tile([128, 512], F32, tag="acc")
                    for i in range(NJ):
                        nc.tensor.matmul(acc,
                                         s_sb[i][:, t * 128:(t + 1) * 128],
                                         wv_sb[i][:, n * 512:(n + 1) * 512],
                                         start=(i == 0), stop=(i == NJ - 1))
                    nc.scalar.activation(vt[:, n * 512:(n + 1) * 512], acc,
                                         Copy)
                v_sb.append(vt)

            # prefetch next chunk's src while attention runs
            if ci + 1 < nchunk:
                s_next = load_src_chunk(ci + 1)

            # ---- per pair: attention for 2 head-groups, then out-proj ----
            for p in range(pairs):
                pc = p * 128
                attn_t = attnp.tile([128, NJ, 128], BF16, tag=f"attn{p % 2}")
                for hg in range(H // 4):
                    sc = ps_sc.tile([128, 512], F32, tag="sc")
                    nc.tensor.matmul(sc, maskl_sb, maskr_sb,
                                     start=True, stop=False)
                    for hh in range(4):
                        h = hg * 4 + hh
                        hs = slice(hh * 128, (hh + 1) * 128)
                        nc.tensor.matmul(sc[:, hs],
                                         qt_sb[h][:, pc:pc + 128],
                                         kt_sb[h][:, pc:pc + 128],
                                         start=False, stop=(hh == 3))
                    exp_sb = smp.tile([128, 4, 128], F32, tag="exp")
                    nc.scalar.activation(exp_sb,
                                         sc.rearrange("p (a b) -> p a b", a=4),
                                         Exp, scale=SCALE)
                    den = smp.tile([128, 4], F32, tag="den")
                    nc.vector.reduce_sum(den, exp_sb,
                                         axis=mybir.AxisListType.X)
                    rcp = smp.tile([128, 4], F32, tag="rcp")
                    nc.vector.reciprocal(rcp, den)
                    probs = smp.tile([128, 4, 128], BF16, tag="probs")
                    nc.vector.tensor_tensor(
                        probs, exp_sb,
                        rcp.rearrange("p (a o) -> p a o", o=1).broadcast_to((128, 4, 128)),
                        op=Mult)
                    probsT = smp.tile([128, 512], BF16, tag="probsT")
                    if USE_DMA_TR:
                        for hh in range(4):
                            hs = slice(hh * 128, (hh + 1) * 128)
                            nc.sync.dma_start_transpose(probsT[:, hs],
                                                        probs[:, hh, :])
                    else:
                        trp = ps_tr.tile([128, 512], BF16, tag="trp")
                        for hh in range(4):
                            hs = slice(hh * 128, (hh + 1) * 128)
                            nc.tensor.transpose(trp[:, hs], probs[:, hh, :],
                                                ident_sb)
                        nc.scalar.activation(probsT, trp, Copy)
                    ao = ps_ao.tile([128, 512], F32, tag="ao")
                    for hh in range(4):
                        h = hg * 4 + hh
                        hs = slice(hh * 128, (hh + 1) * 128)
                        nc.tensor.matmul(ao[:, hs],
                                         v_sb[p][:, h * 128:(h + 1) * 128],
                                         probsT[:, hs], start=True, stop=True)
                    nc.scalar.activation(attn_t[:, hg * 4:(hg + 1) * 4, :],
                                         ao.rearrange("p (a b) -> p a b", a=4),
                                         Copy)

                # ---- output projection for this pair (token-major) ----
                o_sb = outp.tile([128, D], F32, tag="o")
                for n in range(nhalf):
                    acc = ps_proj.tile([128, 512], F32, tag="acc")
                    for i in range(NJ):
                        nc.tensor.matmul(acc, attn_t[:, i, :],
                                         wo_sb[i][:, n * 512:(n + 1) * 512],
                                         start=(i == 0), stop=(i == NJ - 1))
                    nc.vector.tensor_add(o_sb[:, n * 512:(n + 1) * 512], acc,
                                         bo_sb[:, n * 512:(n + 1) * 512])
                nc.sync.dma_start(out=out[c0 + p * 128:c0 + (p + 1) * 128, :],
                                  in_=o_sb)

    nc.compile()
    return nc


def make_host_inputs(src, Wq, bq, Wk, bk, Wv, bv, Wo, bo, t_core=T_CORE,
                     n_cores=N_CORES):
    """Prepare per-core input maps (host-side shard + transpose + bf16 cast)."""
    bf = ml_dtypes.bfloat16
    tokens = np.ascontiguousarray(np.asarray(src, dtype=np.float32)
                                  .reshape(-1, D))
    srct = np.ascontiguousarray(tokens.T).astype(bf)          # [D, T_total]
    wq16 = np.asarray(Wq, dtype=np.float32).astype(bf)
    wk16 = np.asarray(Wk, dtype=np.float32).astype(bf)
    wv16 = np.asarray(Wv, dtype=np.float32).astype(bf)
    wo16 = np.asarray(Wo, dtype=np.float32).astype(bf)
    bqt = np.ascontiguousarray(np.asarray(bq, np.float32).reshape(NJ, 128).T)
    bkt = np.ascontiguousarray(np.asarray(bk, np.float32).reshape(NJ, 128).T)
    # probs rows sum to 1, so  attn@(Wo) with v-bias folds into the output
    # bias:  out = (attn0 + bv)@Wo + bo = attn0@Wo + (bv@Wo + bo)
    bof = (np.asarray(bo, np.float64)
           + np.asarray(bv, np.float64) @ np.asarray(Wo, np.float64)
           ).astype(np.float32)
    u0 = np.zeros((128,), np.float32); u0[:64] = 1.0
    u1 = np.zeros((128,), np.float32); u1[64:] = 1.0
    maskl = np.stack([u0, u1]).astype(bf)
    maskr1 = np.stack([MASK_RAW * u1, MASK_RAW * u0])
    maskr = np.tile(maskr1, (1, 4)).astype(bf)                # [2, 512]
    ident = np.eye(128, dtype=np.float32).astype(bf)
    in_maps = []
    for c in range(n_cores):
        in_maps.append({
            "srct": np.ascontiguousarray(srct[:, c * t_core:(c + 1) * t_core]),
            "wq": wq16, "wk": wk16, "wv": wv16, "wo": wo16,
            "bqt": bqt, "bkt": bkt, "bo": bof,
            "maskl": maskl, "maskr": maskr, "ident": ident,
        })
    return in_maps


_NC_CACHE = {}


def _get_nc():
    if "nc" not in _NC_CACHE:
        _NC_CACHE["nc"] = build_nc()
    return _NC_CACHE["nc"]


def run_on_hw(in_maps, **kwargs):
    nc = _get_nc()
    return run_bass_kernel_spmd(nc, in_maps, core_ids=list(range(N_CORES)),
                                **kwargs)


def kernel(src, Wq, bq, Wk, bk, Wv, bv, Wo, bo):
    in_maps = make_host_inputs(src, Wq, bq, Wk, bk, Wv, bv, Wo, bo)
    res = run_on_hw(in_maps)
    out = np.concatenate([res.results[c]["out"] for c in range(N_CORES)],
                         axis=0)
    return out.reshape(B, S, D).astype(np.float32)
